# revision 5
# baseline (speedup 1.0000x reference)
"""LSTM autoencoder (2-layer enc + autoregressive 2-layer dec + fc) on 8 trn2 cores.

Pure batch data-parallel: 4096 -> 512/core = 2 chains x 4 groups x 64 batch
(free dim 64). Gates on partitions, quadrant q in [I,O,G,F]; encoder packs
2 layers x 4 groups x 4 units = 32 rows/quadrant with the layer-1 lane
staggered one step behind layer 0 so both layers share one matmul + one act
per step; decoder runs its two cells serially (the fc feedback makes them
inseparable).

All gate activations via one Tanh act (sigmoid(x) = (tanh(x/2)+1)/2); h/c
stored doubled (hs=2h, cs=2c at base 96) with 0.5 folded into host-built
block-diagonal weights. Matmul operands are float32r (single-issue weight
load, 2 cyc/row); PSUM accumulates fp32; elementwise stays fp32.

X enters as a second accumulating matmul issued *before* the recurrent one:
each 16-step window is bulk-transposed into (t%4)-phase blocks
[4t x 4g x 8f, 64b]; the phase selects the 32-row partition base of both
the replicated weights and the staged rhs, so no per-step reassembly is
needed. Outputs transpose back through PSUM in 16-step windows, time-reversed.

Engine placement (found by timeline search): acts + stage/flush copies +
y-bias on Act; u/v/csn/hs/rh on DVE; the cross-quadrant gate copy on Pool
(GPSIMD cannot touch PSUM); all DMAs on SP.
"""

import numpy as np
import ml_dtypes
from contextlib import ExitStack

import concourse.bass as bass
import concourse.bacc as bacc
import concourse.tile as tile
import concourse.mybir as mybir
from concourse.bass_utils import run_bass_kernel_spmd

DT = mybir.dt.float32
BF = mybir.dt.float32r
BH = mybir.dt.float32  # elementwise dtype (bf16 gave no speedup)
AF = mybir.ActivationFunctionType
AO = mybir.AluOpType

B, T, F, H = 4096, 1024, 8, 4
NCORES = 8
BC = B // NCORES          # 512 batch per core
S = 2                     # chains per core
NGc = 4                   # groups per chain
PB = 64                   # batch per group (free dim)
TW = 16                   # timesteps per input/output window
GATE_PY = [0, 3, 2, 1]    # quadrant q -> pytorch gate row-block (i,f,g,o order)

bf16 = ml_dtypes.bfloat16


def _g(w, q):
    p = GATE_PY[q]
    return w[4 * p:4 * p + 4]


def build_consts(inp):
    f32 = np.float32
    eWih0, eWhh0, eb0 = inp["enc_Wih0"], inp["enc_Whh0"], inp["enc_b0"]
    eWih1, eWhh1, eb1 = inp["enc_Wih1"], inp["enc_Whh1"], inp["enc_b1"]
    dWih0, dWhh0, db0 = inp["dec_Wih0"], inp["dec_Whh0"], inp["dec_b0"]
    dWih1, dWhh1, db1 = inp["dec_Wih1"], inp["dec_Whh1"], inp["dec_b1"]
    fcW, fcb = inp["fc_W"], inp["fc_b"]

    # encoder: psum col m = q*32 + L*16 + g*4 + u
    Wh_e = np.zeros((32, 128), f32)    # rows: L*16 + g*4 + u_in
    Wx_e = np.zeros((128, 128), f32)   # rows: 32*p + g*8 + f  (4 phases)
    asc_e = np.ones((128, 1), f32)
    abe = np.zeros((128, 1), f32)
    abe0 = np.zeros((128, 1), f32)     # step-0 bias: forces L1 lane to zero
    for q in range(4):
        w0h, w1i = _g(eWhh0, q), _g(eWih1, q)
        w1h, w0x = _g(eWhh1, q), _g(eWih0, q)
        b0 = _g(eb0[:, None], q)[:, 0]
        b1 = _g(eb1[:, None], q)[:, 0]
        sc = 0.5 if q != 2 else 1.0
        for g in range(4):
            for u in range(4):
                m0 = q * 32 + g * 4 + u
                m1 = q * 32 + 16 + g * 4 + u
                asc_e[m0, 0] = sc
                asc_e[m1, 0] = sc
                abe[m0, 0] = sc * b0[u]
                abe[m1, 0] = sc * b1[u]
                abe0[m0, 0] = sc * b0[u]
                abe0[m1, 0] = -60.0
                for ui in range(H):
                    Wh_e[g * 4 + ui, m0] = 0.5 * w0h[u, ui]
                    Wh_e[g * 4 + ui, m1] = 0.5 * w1i[u, ui]
                    Wh_e[16 + g * 4 + ui, m1] = 0.5 * w1h[u, ui]
                for f in range(F):
                    for p in range(4):
                        Wx_e[32 * p + g * 8 + f, m0] = w0x[u, f]

    # decoder: psum col m = q*32 + g*4 + u
    wcomp = dWih0 @ (0.5 * fcW)
    bshift = dWih0 @ fcb
    W0h = np.zeros((16, 128), f32)
    W0x = np.zeros((16, 128), f32)
    W1h = np.zeros((16, 128), f32)
    W1x = np.zeros((16, 128), f32)
    ascd = np.ones((128, 1), f32)
    abd0 = np.zeros((128, 1), f32)
    abd0f = np.zeros((128, 1), f32)
    abd1 = np.zeros((128, 1), f32)
    for q in range(4):
        wh0, wx = _g(dWhh0, q), _g(wcomp, q)
        w1i, w1h = _g(dWih1, q), _g(dWhh1, q)
        b0 = _g(db0[:, None], q)[:, 0]
        bs = _g(bshift[:, None], q)[:, 0]
        b1 = _g(db1[:, None], q)[:, 0]
        sc = 0.5 if q != 2 else 1.0
        for g in range(4):
            for u in range(4):
                m = q * 32 + g * 4 + u
                ascd[m, 0] = sc
                abd0f[m, 0] = sc * b0[u]
                abd0[m, 0] = sc * (b0[u] + bs[u])
                abd1[m, 0] = sc * b1[u]
                for ui in range(H):
                    W0h[g * 4 + ui, m] = 0.5 * wh0[u, ui]
                    W0x[g * 4 + ui, m] = wx[u, ui]
                    W1h[g * 4 + ui, m] = 0.5 * w1h[u, ui]
                    W1x[g * 4 + ui, m] = 0.5 * w1i[u, ui]

    wfc = np.zeros((16, 32), f32)
    fcbv = np.zeros((32, 1), f32)
    for g in range(4):
        for f in range(F):
            fcbv[g * 8 + f, 0] = fcb[f]
            for u in range(H):
                wfc[g * 4 + u, g * 8 + f] = 0.5 * fcW[f, u]

    bfc = lambda x: np.ascontiguousarray(x.astype(f32))
    return {
        "Wh_e": bfc(Wh_e), "Wx_e": bfc(Wx_e),
        "W0h": bfc(W0h), "W0x": bfc(W0x), "W1h": bfc(W1h), "W1x": bfc(W1x),
        "wfc": bfc(wfc),
        "asc_e": asc_e, "abe": abe, "abe0": abe0,
        "ascd": ascd, "abd0": abd0, "abd0f": abd0f, "abd1": abd1,
        "fcbv": fcbv,
        "id64": np.eye(64, dtype=f32), "id32": np.eye(32, dtype=f32),
    }


CONST_SPECS = {
    "Wh_e": ((32, 128), BF), "Wx_e": ((128, 128), BF),
    "W0h": ((16, 128), BF), "W0x": ((16, 128), BF),
    "W1h": ((16, 128), BF), "W1x": ((16, 128), BF),
    "wfc": ((16, 32), BF),
    "asc_e": ((128, 1), DT), "abe": ((128, 1), DT), "abe0": ((128, 1), DT),
    "ascd": ((128, 1), DT), "abd0": ((128, 1), DT), "abd0f": ((128, 1), DT),
    "abd1": ((128, 1), DT),
    "fcbv": ((32, 1), DT),
    "id64": ((64, 64), DT), "id32": ((32, 32), BF),
}


def build_nc(Tl=T):
    nc = bacc.Bacc("TRN2", target_bir_lowering=False, debug=False)
    Xd = nc.dram_tensor("x", [BC, Tl, F], DT, kind="ExternalInput")
    Yd = nc.dram_tensor("y", [BC, Tl, F], DT, kind="ExternalOutput")
    cdram = {k: nc.dram_tensor(k, list(s), dt, kind="ExternalInput")
             for k, (s, dt) in CONST_SPECS.items()}

    NW = Tl // TW
    assert Tl % TW == 0

    with tile.TileContext(nc) as tc, ExitStack() as ctx:
        p = lambda name, bufs, **kw: ctx.enter_context(
            tc.tile_pool(name=name, bufs=bufs, **kw))
        wsb = p("wsb", 1)
        xswp = p("xsw", 2 * S)                 # window loads
        stp = p("st", 8 * S)                   # staged x-transposes (bf16)
        psT = p("psT", 2, space="PSUM")
        psZ = p("psZ", 3, space="PSUM")
        psY = p("psY", 1, space="PSUM")
        psO = p("psO", 2, space="PSUM")
        tgp = p("tg", 4)
        gcp = p("gc", 4)
        up = p("u", 4)
        vp = p("v", 4)
        csp = p("cs", 6)
        tcp = p("tc", 4)
        hsp = p("hs", 6)
        rhp_pool = p("rh", 4)
        ytp = p("yt", 4)
        obp = p("ob", 4)

        csb = {}
        for k, (s, dt) in CONST_SPECS.items():
            t_ = wsb.tile(list(s), dt, name=f"c_{k}")
            nc.sync.dma_start(t_[:, :], cdram[k].ap()[:, :])
            csb[k] = t_

        Xv = Xd.ap().rearrange("(n b) t f -> n b t f", n=S * NGc)

        def load_window(w, stage_cur):
            n0 = w * TW
            for c in range(S):
                xsw = xswp.tile([PB, TW * NGc * F], DT, name="xsw")
                xswv = xsw[:, :].rearrange("b (t g f) -> b t g f", g=NGc, f=F)
                for j in range(NGc):
                    nc.sync.dma_start(
                        xswv[:, :, j, :],
                        Xv[NGc * c + j, :, n0:n0 + TW, :])
                xv4 = xsw[:, :].rearrange("b (t g f) -> b t g f", g=NGc, f=F)
                for q in range(TW // 4):
                    pT = psT.tile([128, PB], DT, name="pT")
                    nc.tensor.matmul(pT[:, :], xv4[:, 4 * q:4 * q + 4, :, :],
                                     csb["id64"][:, :], is_transpose=True)
                    st = stp.tile([128, PB], BF, name="st")
                    nc.scalar.copy(st[:, :], pT[:, :])
                    stage_cur[c][q] = st

        # ---------------- encoder ----------------
        hs_prev, cs_prev = [None] * S, [None] * S
        for c in range(S):
            cs_prev[c] = csp.tile([128, PB], BH, name="cs")
            nc.vector.memset(cs_prev[c][96:128, :], 0.0)

        stage_cur = [[None] * (TW // 4) for _ in range(S)]
        enc_h2, enc_c2, enc_h1, enc_c1 = [None] * S, [None] * S, [None] * S, [None] * S

        for n in range(Tl + 1):
            if n < Tl and n % TW == 0:
                load_window(n // TW, stage_cur)
            ph, blk = n % 4, (n % TW) // 4
            pzs, tgs = [None] * S, [None] * S
            for c in range(S):
                pz = psZ.tile([128, PB], DT, name="pz")
                if n < Tl:
                    st = stage_cur[c][blk]
                    nc.tensor.matmul(pz[:, :],
                                     csb["Wx_e"][32 * ph:32 * ph + 32, :],
                                     st[32 * ph:32 * ph + 32, :],
                                     start=True, stop=(n == 0),
                                     tile_position=(32 * ph, 0))
                if n > 0:
                    nc.tensor.matmul(pz[:, :], csb["Wh_e"][:, :],
                                     hs_prev[c][:, :],
                                     start=(n == Tl), stop=True)
                pzs[c] = pz
            bias = csb["abe0"] if n == 0 else csb["abe"]
            for c in range(S):
                tg = tgp.tile([128, PB], BH, name="tg")
                nc.scalar.activation(tg[:, :], pzs[c][:, :], AF.Tanh,
                                     bias=bias[:, 0:1], scale=csb["asc_e"][:, 0:1])
                tgs[c] = tg
            gcs, vs, us = [None] * S, [None] * S, [None] * S
            for c in range(S):
                gc = gcp.tile([32, PB], BH, name="gc")
                nc.gpsimd.tensor_copy(gc[:, :], tgs[c][64:96, :])
                gcs[c] = gc
                v = vp.tile([32, PB], BH, name="v")
                nc.vector.scalar_tensor_tensor(
                    v[:, :], tgs[c][96:128, :], 1.0, cs_prev[c][96:128, :],
                    AO.add, AO.mult)
                vs[c] = v
            for c in range(S):
                u = up.tile([32, PB], BH, name="u")
                nc.vector.scalar_tensor_tensor(
                    u[:, :], tgs[c][0:32, :], 1.0, gcs[c][:, :], AO.add, AO.mult)
                us[c] = u
            csns, tcns = [None] * S, [None] * S
            for c in range(S):
                csn = csp.tile([128, PB], BH, name="cs")
                nc.vector.scalar_tensor_tensor(
                    csn[96:128, :], vs[c][:, :], 0.5, us[c][:, :],
                    AO.mult, AO.add)
                csns[c] = csn
            for c in range(S):
                tcn = tcp.tile([64, PB], BH, name="tc")
                nc.scalar.activation(tcn[32:64, :], csns[c][96:128, :],
                                     AF.Tanh, bias=0.0, scale=0.5)
                tcns[c] = tcn
            for c in range(S):
                hsn = hsp.tile([32, PB], BF, name="hs")
                nc.vector.scalar_tensor_tensor(
                    hsn[:, :], tgs[c][32:64, :], 1.0, tcns[c][32:64, :],
                    AO.add, AO.mult)
                if n == Tl - 1:
                    enc_h2[c], enc_c2[c] = hsn, csns[c]
                if n == Tl:
                    enc_h1[c], enc_c1[c] = hsn, csns[c]
                hs_prev[c], cs_prev[c] = hsn, csns[c]

        # ---------------- decoder init ----------------
        h0p, c0p, h1p, c1p, rhv = [None] * S, [None] * S, [None] * S, [None] * S, [None] * S
        for c in range(S):
            h0p[c] = hsp.tile([16, PB], BF, name="hs")
            nc.vector.tensor_copy(h0p[c][:, :], enc_h2[c][0:16, :])
            c0p[c] = csp.tile([128, PB], BH, name="cs")
            nc.vector.tensor_copy(c0p[c][96:112, :], enc_c2[c][96:112, :])
            h1p[c] = hsp.tile([16, PB], BF, name="hs")
            nc.sync.dma_start(h1p[c][:, :], enc_h1[c][16:32, :])
            c1p[c] = csp.tile([128, PB], BH, name="cs")
            nc.sync.dma_start(c1p[c][96:112, :], enc_c1[c][112:128, :])

        # ---------------- decoder ----------------
        def cell(wh, h_rec, wx, x_in, cs_rec, bias_ap):
            """Emit one LSTM cell for all chains; returns (hs_list, cs_list).
            wh/wx: const names; h_rec/x_in/cs_rec: per-chain tiles."""
            pzs, tgs = [None] * S, [None] * S
            for c in range(S):
                pz = psZ.tile([128, PB], DT, name="pz")
                nc.tensor.matmul(pz[:, :], csb[wh][:, :], h_rec[c][:, :],
                                 start=True, stop=(x_in is None))
                if x_in is not None:
                    nc.tensor.matmul(pz[:, :], csb[wx][:, :], x_in[c][:, :],
                                     start=False, stop=True)
                pzs[c] = pz
            for c in range(S):
                tg = tgp.tile([128, PB], BH, name="tg")
                nc.scalar.activation(tg[:, :], pzs[c][:, :], AF.Tanh,
                                     bias=bias_ap[:, 0:1], scale=csb["ascd"][:, 0:1])
                tgs[c] = tg
            gcs, vs, us = [None] * S, [None] * S, [None] * S
            for c in range(S):
                gc = gcp.tile([16, PB], BH, name="gc")
                nc.gpsimd.tensor_copy(gc[:, :], tgs[c][64:80, :])
                gcs[c] = gc
                v = vp.tile([16, PB], BH, name="v")
                nc.vector.scalar_tensor_tensor(
                    v[:, :], tgs[c][96:112, :], 1.0, cs_rec[c][96:112, :],
                    AO.add, AO.mult)
                vs[c] = v
            for c in range(S):
                u = up.tile([16, PB], BH, name="u")
                nc.vector.scalar_tensor_tensor(
                    u[:, :], tgs[c][0:16, :], 1.0, gcs[c][:, :], AO.add, AO.mult)
                us[c] = u
            csns, tcns, hsns = [None] * S, [None] * S, [None] * S
            for c in range(S):
                csn = csp.tile([128, PB], BH, name="cs")
                nc.vector.scalar_tensor_tensor(
                    csn[96:112, :], vs[c][:, :], 0.5, us[c][:, :],
                    AO.mult, AO.add)
                csns[c] = csn
            for c in range(S):
                tcn = tcp.tile([64, PB], BH, name="tc")
                nc.scalar.activation(tcn[32:48, :], csns[c][96:112, :],
                                     AF.Tanh, bias=0.0, scale=0.5)
                tcns[c] = tcn
            for c in range(S):
                hsn = hsp.tile([16, PB], BF, name="hs")
                nc.vector.scalar_tensor_tensor(
                    hsn[:, :], tgs[c][32:48, :], 1.0, tcns[c][32:48, :],
                    AO.add, AO.mult)
                hsns[c] = hsn
            return hsns, csns

        psO_cur = [None] * S
        for t in range(Tl):
            if t % TW == 0:
                for c in range(S):
                    psO_cur[c] = psO.tile([64, TW * 32], BF, name="psO")
            jblk = TW - 1 - (t % TW)

            bias0 = csb["abd0f"] if t == 0 else csb["abd0"]
            h0n, c0n = cell("W0h", h0p, "W0x", None if t == 0 else rhv,
                            c0p, bias0)
            h1n, c1n = cell("W1h", h1p, "W1x", h0n, c1p, csb["abd1"])
            rhn = [None] * S
            for c in range(S):
                rh = rhp_pool.tile([16, PB], BF, name="rh")
                nc.vector.tensor_scalar_max(rh[:, :], h1n[c][:, :], 0.0)
                rhn[c] = rh
            for c in range(S):
                py = psY.tile([32, PB], DT, name="py")
                nc.tensor.matmul(py[:, :], csb["wfc"][:, :], rhn[c][:, :],
                                 start=True, stop=True)
                yt = ytp.tile([32, PB], BF, name="yt")
                nc.scalar.add(yt[:, :], py[:, :], csb["fcbv"][:, 0:1])
                nc.tensor.matmul(
                    psO_cur[c][:, jblk * 32:(jblk + 1) * 32],
                    yt[:, :], csb["id32"][:, :], is_transpose=True)
            h0p, c0p, h1p, c1p, rhv = h0n, c0n, h1n, c1n, rhn

            if t % TW == TW - 1:
                base = Tl - TW * (t // TW + 1)
                for c in range(S):
                    src = psO_cur[c][:, :].rearrange(
                        "p (t g f) -> p t g f", g=NGc, f=F)
                    for j in range(NGc):
                        ob = obp.tile([PB, TW * F], DT, name="ob")
                        nc.scalar.copy(
                            ob[:, :].rearrange("p (t f) -> p t f", f=F),
                            src[:, :, j, :])
                        gb = (c * NGc + j) * PB
                        nc.sync.dma_start(
                            Yd.ap()[gb:gb + PB, base:base + TW, :],
                            ob[:, :].rearrange("p (t f) -> p t f", f=F))
    nc.compile()
    return nc


_NC_CACHE = {}


def get_nc(Tl=T):
    if Tl not in _NC_CACHE:
        _NC_CACHE[Tl] = build_nc(Tl)
    return _NC_CACHE[Tl]


def kernel(**inputs):
    X = np.ascontiguousarray(np.asarray(inputs["X"], dtype=np.float32))
    Tl = X.shape[1]
    consts = build_consts({k: np.asarray(v, dtype=np.float32)
                           for k, v in inputs.items() if k != "X"})
    nc = get_nc(Tl)
    in_maps = []
    for core in range(NCORES):
        m = {"x": X[core * BC:(core + 1) * BC]}
        m.update(consts)
        in_maps.append(m)
    res = run_bass_kernel_spmd(nc, in_maps, core_ids=list(range(NCORES)))
    out = np.concatenate([r["y"] for r in res.results], axis=0)
    return out.astype(np.float32)
